# revision 19
# baseline (speedup 1.0000x reference)
"""Multi-head attention (B=4, S=2048, D=1024, H=16) on 8 Trainium2 cores.

Sharding: core = (batch b, head-group g) with 4 batches x 2 groups of 8 heads.
Each core computes, for its batch and its 8 heads:
  QT = (x_q @ Wq_g^T)^T            [512, S]   (feature-major, bf16)
  KT = (x_k @ Wk_g^T)^T            [512, S]   (feature-major, bf16)
  V' =  x_v @ Wv_g^T               [S, 8, 65] (seq-major, + ones column/head)
  per (head-pair, k-tile): scoresT[k, q] on PE, exp on ACT -> probs bf16
  attnU[q, 65] += probs_tile^T-stationary x V' (per head, per q-tile of 128)
    -> column 64 = softmax denominator (per-partition scalar!)
  normalize on DVE (tensor_scalar with per-partition reciprocal)
  PE-transpose pairs [128q, 128f] -> [128f, 128q] -> attnT
  outT_partial = woT^T-contraction over the 512 local features  [D, S]
Host: per batch, sum the two groups' outT partials, transpose, add b_o.

Key cost-model facts driving the design (TRN2):
  matmul cost = out-free-size x pe_cycle, independent of M and K
  -> attnV in [q, 65] orientation costs 65 cycles/instr instead of 512
  -> denominators ride along as the ones-column (no separate ones-matmuls)
  -> normalization is per-partition, so no PE broadcast matmuls
K/V/Q/O projection work is interleaved ("fillers") into the first q-chunk's
attention steps so the ACT engine (exp, the second-busiest engine) starts
early and stays fed.
"""

import os
from collections import deque

import ml_dtypes
import numpy as np

import concourse.bass as bass
import concourse.mybir as mybir
import concourse.tile as tile
from concourse import bacc

B = 4
S = 2048
D = 1024
H = 16
DK = 64
NCORES = 8
GROUPS = 2
HPC = H // GROUPS  # heads per core (8)
FC = HPC * DK  # local features per core (512)
P = 128

F32 = mybir.dt.float32
F32R = mybir.dt.float32r
BF16 = mybir.dt.bfloat16

_NC_CACHE = {}


def build_nc(s=S, d=D, hpc=HPC, bias=False, nq=512):
    fc = hpc * DK
    ndt = d // P  # contraction tiles for projections (8)
    nft = fc // P  # local feature tiles (4) == head pairs
    nqc = s // nq  # q chunks (4)
    nkt = s // P  # k tiles (16)
    nqt = nq // P  # q subtiles of 128 per chunk (4)
    VW = DK + 1  # V width per head incl ones column
    inv_sqrt_dk = 1.0 / float(np.sqrt(DK))

    PEND = int(os.environ.get("PEND", "2"))
    ETB = int(os.environ.get("ETB", "24"))
    XB = int(os.environ.get("XB", "2"))

    nc = bacc.Bacc("TRN2", target_bir_lowering=False, debug=False)

    vdt = F32 if bias else BF16
    xqT = nc.dram_tensor("xqT", [d, s], vdt, kind="ExternalInput").ap()
    xkT = nc.dram_tensor("xkT", [d, s], vdt, kind="ExternalInput").ap()
    xvT = nc.dram_tensor("xvT", [d, s], vdt, kind="ExternalInput").ap()
    wqT = nc.dram_tensor("wqT", [d, fc], vdt, kind="ExternalInput").ap()
    wkT = nc.dram_tensor("wkT", [d, fc], vdt, kind="ExternalInput").ap()
    wvT = nc.dram_tensor("wvT", [d, fc], vdt, kind="ExternalInput").ap()
    woT = nc.dram_tensor("woT", [fc, d], BF16, kind="ExternalInput").ap()
    outT = nc.dram_tensor("outT", [d, s], F32, kind="ExternalOutput").ap()
    if bias:
        bq = nc.dram_tensor("bq", [1, fc], F32, kind="ExternalInput").ap()
        bk = nc.dram_tensor("bk", [1, fc], F32, kind="ExternalInput").ap()
        bv = nc.dram_tensor("bv", [1, fc], F32, kind="ExternalInput").ap()
        ones_dram = nc.inline_tensor(
            np.ones((1, max(nq, P)), np.float32), name="ones_const"
        ).ap()

    idn_dram = nc.inline_tensor(
        np.eye(P, dtype=ml_dtypes.bfloat16), name="idn_const"
    ).ap()

    with tile.TileContext(nc) as tc:
        with (
            tc.tile_pool(name="sb", bufs=1) as sb,
            tc.tile_pool(name="ps", bufs=1, space="PSUM") as ps,
        ):
            qt_t = sb.tile([P, nft, s], BF16, tag="QT")
            kt_t = sb.tile([P, nft, s], BF16, tag="KT")
            vp_t = sb.tile([P, nkt, hpc, VW], BF16, tag="Vp")
            wq_t = sb.tile([P, ndt, fc], vdt, tag="wq")
            wk_t = sb.tile([P, ndt, fc], vdt, tag="wk")
            wv_t = sb.tile([P, ndt, fc], vdt, tag="wv")
            wo_t = sb.tile([P, nft, d], BF16, tag="wo")
            idn_t = sb.tile([P, P], BF16, tag="idn")

            nc.sync.dma_start(out=idn_t[:], in_=idn_dram[:])
            if bias:
                ones_t = sb.tile([1, max(nq, P)], F32, tag="ones")
                nc.sync.dma_start(out=ones_t[:], in_=ones_dram[:])
                bq_t = sb.tile([1, fc], F32, tag="bq")
                bk_t = sb.tile([1, fc], F32, tag="bk")
                bv_t = sb.tile([1, fc], F32, tag="bv")
                nc.sync.dma_start(out=bq_t[:], in_=bq[:])
                nc.sync.dma_start(out=bk_t[:], in_=bk[:])
                nc.sync.dma_start(out=bv_t[:], in_=bv[:])

            def dma2(dst, src_ap, n):
                # two halves on the two issuing engines (parallel DMA queues)
                h = n // 2
                nc.sync.dma_start(out=dst[:, 0:h], in_=src_ap[:, 0:h])
                nc.gpsimd.dma_start(out=dst[:, h:n], in_=src_ap[:, h:n])

            # loads are emitted in CONSUMPTION order (wk/xk first) so the
            # first K-projection chunk isn't stuck behind unrelated loads
            dma2(wk_t, wkT.rearrange("(t p) f -> p t f", p=P), ndt)

            # ones columns for the softmax denominators (written once;
            # V-proj copies only touch cols 0:DK)
            nc.vector.memset(vp_t[:, :, :, DK : DK + 1], 1.0)

            # x chunk pools (bf16 feature-major inputs)
            def load_x_chunk(src, sl, tag, bufs):
                x_t = sb.tile([P, ndt, sl.stop - sl.start], vdt, tag=tag, bufs=bufs)
                dma2(x_t, src[:, sl].rearrange("(t p) s -> p t s", p=P), ndt)
                return x_t

            # ---- projection emitters (chunked so they can interleave) ----
            def kq_proj_chunk(x_t, dst, ft, qsl, bias_t=None):
                # dst[:, ft, qsl] = (W^T x)[ft tile]  (feature-major)
                w_t = wk_t if dst is kt_t else wq_t
                acc = ps.tile([P, nq], F32, tag="sc", bufs=2)
                n = qsl.stop - qsl.start
                if bias_t is not None:
                    nc.tensor.matmul(
                        acc[:, 0:n],
                        lhsT=bias_t[0:1, ft * P : (ft + 1) * P],
                        rhs=ones_t[0:1, 0:n],
                        start=True,
                        stop=False,
                    )
                for dt in range(ndt):
                    nc.tensor.matmul(
                        acc[:, 0:n],
                        lhsT=w_t[:, dt, ft * P : (ft + 1) * P],
                        rhs=x_t[:, dt, :],
                        start=(dt == 0 and bias_t is None),
                        stop=(dt == ndt - 1),
                    )
                nc.vector.tensor_copy(dst[:, ft, qsl], acc[:, 0:n])

            def v_proj_tile(st):
                xv_t = load_x_chunk(xvT, slice(st * P, (st + 1) * P), "xv", XB)
                acc = ps.tile([P, nq], F32, tag="sc", bufs=2)
                accv = acc[:, 0:fc]
                if bias:
                    nc.tensor.matmul(
                        accv,
                        lhsT=ones_t[0:1, 0:P],
                        rhs=bv_t[0:1, :],
                        start=True,
                        stop=False,
                    )
                for dt in range(ndt):
                    nc.tensor.matmul(
                        accv,
                        lhsT=xv_t[:, dt, :],
                        rhs=wv_t[:, dt, :],
                        start=(dt == 0 and not bias),
                        stop=(dt == ndt - 1),
                    )
                nc.vector.tensor_copy(
                    vp_t[:, st, :, 0:DK],
                    acc[:, 0:fc].rearrange("p (h e) -> p h e", h=hpc),
                )

            def o_proj_chunk(attnT_qc, jt, qsl):
                acc = ps.tile([P, nq], F32, tag="sc", bufs=2)
                for ct in range(nft):
                    nc.tensor.matmul(
                        acc[:],
                        lhsT=wo_t[:, ct, jt * P : (jt + 1) * P],
                        rhs=attnT_qc[:, ct, :],
                        start=(ct == 0),
                        stop=(ct == nft - 1),
                    )
                ot = sb.tile([P, nq], F32, tag="out", bufs=2)
                nc.vector.tensor_copy(ot[:], acc[:])
                nc.gpsimd.dma_start(out=outT[jt * P : (jt + 1) * P, qsl], in_=ot[:])

            # ---- filler scheduling ----------------------------------------
            # fillers: (deadline_step, emit_fn). Steps count (g, kt) pairs
            # globally: step = ((qc*4 + g)*nkt + kt).
            fillers = []

            def emit_due(step):
                while fillers and fillers[0][0] <= step:
                    fillers.pop(0)[1]()

            def flush_fillers():
                while fillers:
                    fillers.pop(0)[1]()

            # ---- prologue: only what the very first scores need ------------
            bk_arg = bk_t if bias else None
            bq_arg = bq_t if bias else None
            xk_tiles = {}
            xk_tiles[0] = load_x_chunk(xkT, slice(0, nq), "xk", 4)
            kq_proj_chunk(xk_tiles[0], kt_t, 0, slice(0, nq), bk_arg)
            dma2(wq_t, wqT.rearrange("(t p) f -> p t f", p=P), ndt)
            xq0 = load_x_chunk(xqT, slice(0, nq), "xq", XB)
            kq_proj_chunk(xq0, qt_t, 0, slice(0, nq), bq_arg)
            dma2(wv_t, wvT.rearrange("(t p) f -> p t f", p=P), ndt)

            # Everything else is a deadline-driven filler inside qc0:
            # K-proj (ft, sc): needed by scores(qc0, g=ft, kt=4sc)
            # Q-proj (qc0, ft): needed by scores(qc0, g=ft, kt=0)
            # V-proj st: needed by attnV(qc0, g0, kt=st), which lags VPEND
            for ft in range(1, nft):
                fillers.append(
                    (nkt * ft - 3, lambda ft=ft: kq_proj_chunk(
                        xq0, qt_t, ft, slice(0, nq), bq_arg))
                )
            for sc in range(nqc):
                if sc > 0:
                    def load_xk(sc=sc):
                        xk_tiles[sc] = load_x_chunk(
                            xkT, slice(sc * nq, (sc + 1) * nq), "xk", 4
                        )
                    fillers.append((max(0, 4 * sc - 5), load_xk))
                for ft in range(nft):
                    if (sc, ft) == (0, 0):
                        continue
                    def kchunk(ft=ft, sc=sc):
                        kq_proj_chunk(
                            xk_tiles[sc], kt_t, ft, slice(sc * nq, (sc + 1) * nq),
                            bk_arg,
                        )
                    fillers.append((ft * nkt + 4 * sc - 2, kchunk))
            for st in range(nkt):
                fillers.append((st, lambda st=st: v_proj_tile(st)))
            # wo load once the prologue burst has cleared
            fillers.append((18, lambda: dma2(
                wo_t, woT.rearrange("(t p) j -> p t j", p=P), nft)))
            fillers.sort(key=lambda f: f[0])

            # ---- main attention pipeline ----------------------------------
            # One flat stream over (qc, g, kt) steps. Group g's attnV
            # matmuls are deferred into a "burst" that executes qi-major
            # (PSUM accumulation regions within a tile must be written by
            # CONSECUTIVE start->stop runs; interleaving kt across regions
            # corrupts the accumulation) and is spread one chunk per step
            # across group g+1's steps. The normalize for head-half 0 rides
            # at chunk 8 so the up tiles stay effectively double-buffered.
            attnT_tiles = {}
            carry = deque()  # burst chunk closures from the previous group
            ready_drains = deque()  # drain closures unlocked by burst end

            def normalize_half(up, attn_n, hh):
                rc = sb.tile([P, nqt], F32, tag="rc", bufs=4)
                with nc.allow_low_precision(
                    reason="softmax denominator reciprocal"
                ):
                    nc.vector.reciprocal(rc[:], up[hh][:, :, DK])
                for qi in range(nqt):
                    nc.vector.tensor_scalar_mul(
                        attn_n[:, qi, hh * DK : (hh + 1) * DK],
                        up[hh][:, qi, 0:DK],
                        rc[:, qi : qi + 1],
                    )

            def drain_transpose(attn_n, attnT, g):
                for qi in range(nqt):
                    tr = ps.tile([P, 2 * nq], BF16, tag="sc", bufs=2, name="tr")
                    nc.tensor.transpose(tr[:, 0:P], attn_n[:, qi, :], idn_t[:])
                    nc.vector.tensor_copy(
                        attnT[:, g, qi * P : (qi + 1) * P], tr[:, 0:P]
                    )

            def make_burst(ets, up, attnT, g):
                # 16 chunks; chunk (hh*8 + qi*2 + half) does kt half-range
                # of accumulation region (hh, qi): strictly sequential per
                # PSUM region. attn_n + normalize for hh0 ride at chunk 8.
                attn_n = sb.tile([P, nqt, P], BF16, tag="ann", bufs=2)
                chunks = []
                for hh in range(2):
                    for qi in range(nqt):
                        for half in range(2):
                            def chunk(hh=hh, qi=qi, half=half):
                                if (hh, qi, half) == (1, 0, 0):
                                    normalize_half(up, attn_n, 0)
                                for kt in range(half * 8, half * 8 + 8):
                                    nc.tensor.matmul(
                                        up[hh][:, qi, :],
                                        lhsT=ets[kt][
                                            :,
                                            hh * nq + qi * P : hh * nq
                                            + (qi + 1) * P,
                                        ],
                                        rhs=vp_t[:, kt, 2 * g + hh, :],
                                        start=(kt == 0),
                                        stop=(kt == nkt - 1),
                                    )
                            chunks.append(chunk)

                def d1b():
                    normalize_half(up, attn_n, 1)

                def d2():
                    drain_transpose(attn_n, attnT, g)

                # the drains become schedulable only once the last burst
                # chunk has been EMITTED (they read the accumulated up)
                last = chunks[-1]

                def last_and_arm(last=last):
                    last()
                    ready_drains.append(d1b)
                    ready_drains.append(d2)

                chunks[-1] = last_and_arm
                return chunks

            extra_tasks = deque()  # (due_step, fn) for O/Q-proj ride-alongs

            for qc in range(nqc):
                qsl = slice(qc * nq, (qc + 1) * nq)
                attnT = sb.tile([P, nft, nq], BF16, tag="atn", bufs=3)
                attnT_tiles[qc] = attnT
                if qc < nqc - 1:
                    # Q-proj for qc+1 rides along in this qc's steps
                    nsl = slice((qc + 1) * nq, (qc + 2) * nq)

                    def load_xq(nsl=nsl):
                        load_xq.tile = load_x_chunk(xqT, nsl, "xq", XB)

                    base = qc * nft * nkt
                    extra_tasks.append((base + 20, load_xq))
                    for ft in range(nft):
                        def qchunk(ft=ft, nsl=nsl, load_xq=load_xq):
                            kq_proj_chunk(load_xq.tile, qt_t, ft, nsl, bq_arg)
                        extra_tasks.append((base + 26 + 8 * ft, qchunk))
                if qc > 0:
                    # O-proj for qc-1 rides along
                    prev = attnT_tiles[qc - 1]
                    prev_qsl = slice((qc - 1) * nq, qc * nq)
                    base = qc * nft * nkt
                    for jt in range(d // P):
                        def ochunk(jt=jt, prev=prev, prev_qsl=prev_qsl):
                            o_proj_chunk(prev, jt, prev_qsl)
                        extra_tasks.append((base + 12 + 6 * jt, ochunk))

                for g in range(nft):
                    up = []
                    for hh in range(2):
                        t = ps.tile(
                            [P, nqt, VW], F32, tag="u", bufs=2, name=f"up{hh}"
                        )
                        up.append(t)
                    ets = []
                    for kt in range(nkt):
                        step = (qc * nft + g) * nkt + kt
                        pp = ps.tile([P, 2 * nq], F32, tag="pp", bufs=2)
                        for hh in range(2):
                            rows = slice(hh * DK, (hh + 1) * DK)
                            nc.tensor.matmul(
                                pp[:, hh * nq : (hh + 1) * nq],
                                lhsT=kt_t[rows, g, kt * P : (kt + 1) * P],
                                rhs=qt_t[rows, g, qsl],
                                start=True,
                                stop=True,
                            )
                        et = sb.tile([P, 2 * nq], BF16, tag="exp", bufs=ETB)
                        nc.scalar.activation(
                            et[:],
                            pp[:],
                            mybir.ActivationFunctionType.Exp,
                            scale=inv_sqrt_dk,
                        )
                        ets.append(et)
                        if ready_drains:
                            ready_drains.popleft()()
                        for _ in range(2):
                            if carry:
                                carry.popleft()()
                        emit_due(step)
                        while extra_tasks and extra_tasks[0][0] <= step:
                            extra_tasks.popleft()[1]()
                    assert not carry
                    carry.extend(make_burst(ets, up, attnT, g))

            # tail: emit remaining burst, drains, last O-proj
            while carry:
                carry.popleft()()
            while ready_drains:
                ready_drains.popleft()()
            while extra_tasks:
                extra_tasks.popleft()[1]()
            flush_fillers()
            for jt in range(d // P):
                o_proj_chunk(attnT_tiles[nqc - 1], jt, slice(s - nq, s))

    nc.compile()
    return nc


def _get_nc(bias):
    if bias not in _NC_CACHE:
        _NC_CACHE[bias] = build_nc(bias=bias)
    return _NC_CACHE[bias]


def make_in_maps(query, key_, value, w_q, b_q, w_k, b_k, w_v, b_v, w_o, b_o):
    bias = bool(np.any(b_q) or np.any(b_k) or np.any(b_v))
    pdt = np.float32 if bias else ml_dtypes.bfloat16
    xT = {}
    for b in range(B):
        xT[("q", b)] = np.ascontiguousarray(query[b].T).astype(pdt)
        xT[("k", b)] = np.ascontiguousarray(key_[b].T).astype(pdt)
        xT[("v", b)] = np.ascontiguousarray(value[b].T).astype(pdt)
    wT = {}
    for g in range(GROUPS):
        rows = slice(g * FC, (g + 1) * FC)
        wT[("q", g)] = np.ascontiguousarray(w_q[rows, :].T).astype(pdt)
        wT[("k", g)] = np.ascontiguousarray(w_k[rows, :].T).astype(pdt)
        wT[("v", g)] = np.ascontiguousarray(w_v[rows, :].T).astype(pdt)
        wT[("o", g)] = np.ascontiguousarray(w_o[:, rows].T).astype(ml_dtypes.bfloat16)
    in_maps = []
    for core in range(NCORES):
        b, g = core // GROUPS, core % GROUPS
        m = {
            "xqT": xT[("q", b)],
            "xkT": xT[("k", b)],
            "xvT": xT[("v", b)],
            "wqT": wT[("q", g)],
            "wkT": wT[("k", g)],
            "wvT": wT[("v", g)],
            "woT": wT[("o", g)],
        }
        if bias:
            rows = slice(g * FC, (g + 1) * FC)
            m["bq"] = np.ascontiguousarray(b_q[rows]).reshape(1, FC)
            m["bk"] = np.ascontiguousarray(b_k[rows]).reshape(1, FC)
            m["bv"] = np.ascontiguousarray(b_v[rows]).reshape(1, FC)
        in_maps.append(m)
    return in_maps, bias


def assemble(results, b_o):
    out = np.empty((B, S, D), np.float32)
    for b in range(B):
        acc = results[b * GROUPS]["outT"].copy()
        for g in range(1, GROUPS):
            acc += results[b * GROUPS + g]["outT"]
        out[b] = acc.T
    out += np.asarray(b_o, np.float32)
    return out


def kernel(query, key_, value, w_q, b_q, w_k, b_k, w_v, b_v, w_o, b_o):
    args = [
        np.asarray(a, np.float32)
        for a in (query, key_, value, w_q, b_q, w_k, b_k, w_v, b_v, w_o, b_o)
    ]
    query, key_, value, w_q, b_q, w_k, b_k, w_v, b_v, w_o, b_o = args
    in_maps, bias = make_in_maps(
        query, key_, value, w_q, b_q, w_k, b_k, w_v, b_v, w_o, b_o
    )
    nc = _get_nc(bias)
    from concourse.bass_utils import run_bass_kernel_spmd

    res = run_bass_kernel_spmd(nc, in_maps, list(range(NCORES)))
    return assemble(res.results, b_o)
